# revision 10
# baseline (speedup 1.0000x reference)
"""Causal self-attention on 8 Trainium2 NeuronCores.

Sharding: core c handles batch b = c//2 and heads [(c%2)*8, (c%2)*8+8).
Each core computes the full QKV projection for its head slice, causal
flash-style attention, and the row-parallel w_o partial product. The two
partials per batch are summed on the host (no device collectives).

All PE matmuls run in fp16 (1 cycle/row) with fp32 PSUM accumulation.
Feature-major layouts throughout:
  x^T [D, N]        (host pre-transposed)
  Q^T, K^T [ch, N]  (from GEMM with W stationary, x^T moving)
  V [N, ch] + ones  (from GEMM with x^T stationary, W moving)
  S^T [k, q] = K^T_tile.T @ Q^T  -> exp -> P^T [k, q]
  O^T [ch, q] = (V|1).T @ P^T    (row 64 = softmax denominator)
  y = O^T_norm.T @ W_o           (accumulated over ch tiles)

Causal masking: for diagonal-straddling S^T blocks an extra accumulating
matmul adds -BIG * max(k - q, 0) so exp() underflows to exact zeros.
"""

import numpy as np

B, N, D, H = 4, 2048, 1024, 16
DH = 64
N_CORES = 8
HPC = 8            # heads per core
CH = HPC * DH      # 512 channels per core
SCALE = 1.0 / 8.0  # 1/sqrt(DH)
BIG = 280.0        # SCALE*BIG = 35 >> max |S/8|, exp underflows to 0

_cached = None


def _build_program():
    from contextlib import ExitStack

    import concourse.tile as tile
    from concourse import bacc, mybir

    f16 = mybir.dt.float16
    f32 = mybir.dt.float32
    f32r = mybir.dt.float32r
    Exp = mybir.ActivationFunctionType.Exp
    mult = mybir.AluOpType.mult
    add = mybir.AluOpType.add

    nc = bacc.Bacc(
        "TRN2", target_bir_lowering=False, debug=False, num_devices=N_CORES
    )

    xT_d = nc.dram_tensor("xT", [D, N], f16, kind="ExternalInput").ap()
    wq_d = nc.dram_tensor("wq", [D, CH], f16, kind="ExternalInput").ap()
    wk_d = nc.dram_tensor("wk", [D, CH], f16, kind="ExternalInput").ap()
    wv_d = nc.dram_tensor("wv", [D, CH], f16, kind="ExternalInput").ap()
    wo_d = nc.dram_tensor("wo", [CH, D], f16, kind="ExternalInput").ap()
    bq_d = nc.dram_tensor("bq", [CH, 1], f32, kind="ExternalInput").ap()
    bk_d = nc.dram_tensor("bk", [CH, 1], f32, kind="ExternalInput").ap()
    bv_d = nc.dram_tensor("bvb", [128, CH], f32, kind="ExternalInput").ap()
    bo_d = nc.dram_tensor("bob", [128, D], f32, kind="ExternalInput").ap()
    U_d = nc.dram_tensor("U", [128, 128], f16, kind="ExternalInput").ap()
    R_d = nc.dram_tensor("R", [128, 896], f16, kind="ExternalInput").ap()
    y_d = nc.dram_tensor("y", [N, D], f32, kind="ExternalOutput").ap()

    with tile.TileContext(nc) as tc, ExitStack() as ctx:
        const = ctx.enter_context(tc.tile_pool(name="const", bufs=1))
        actp = ctx.enter_context(tc.tile_pool(name="actp", bufs=1))
        work = ctx.enter_context(tc.tile_pool(name="work", bufs=3))
        ps_s = ctx.enter_context(tc.tile_pool(name="ps_s", bufs=2, space="PSUM"))
        ps_p = ctx.enter_context(tc.tile_pool(name="ps_p", bufs=4, space="PSUM"))

        # ---- constants / weights into SBUF ----
        xt = [const.tile([128, N], f16, tag=f"xt{i}", name=f"xt{i}") for i in range(8)]
        for i in range(8):
            nc.sync.dma_start(xt[i][:], xT_d[i * 128 : (i + 1) * 128, :])
        wq = [const.tile([128, CH], f16, tag=f"wq{i}", name=f"wq{i}") for i in range(8)]
        wk = [const.tile([128, CH], f16, tag=f"wk{i}", name=f"wk{i}") for i in range(8)]
        wv = [const.tile([128, CH], f16, tag=f"wv{i}", name=f"wv{i}") for i in range(8)]
        for i in range(8):
            nc.sync.dma_start(wq[i][:], wq_d[i * 128 : (i + 1) * 128, :])
            nc.sync.dma_start(wk[i][:], wk_d[i * 128 : (i + 1) * 128, :])
            nc.sync.dma_start(wv[i][:], wv_d[i * 128 : (i + 1) * 128, :])
        wo = [const.tile([128, D], f16, tag=f"wo{j}", name=f"wo{j}") for j in range(4)]
        for j in range(4):
            nc.sync.dma_start(wo[j][:], wo_d[j * 128 : (j + 1) * 128, :])
        bq = [const.tile([128, 1], f32, tag=f"bq{j}", name=f"bq{j}") for j in range(4)]
        bk = [const.tile([128, 1], f32, tag=f"bk{j}", name=f"bk{j}") for j in range(4)]
        for j in range(4):
            nc.sync.dma_start(bq[j][:], bq_d[j * 128 : (j + 1) * 128, :])
            nc.sync.dma_start(bk[j][:], bk_d[j * 128 : (j + 1) * 128, :])
        bv_t = const.tile([128, CH], f32, tag="bvb", name="bvb")
        nc.sync.dma_start(bv_t[:], bv_d[:])
        bo_t = const.tile([128, D], f32, tag="bob", name="bob")
        nc.sync.dma_start(bo_t[:], bo_d[:])
        U_t = const.tile([128, 128], f16, tag="U", name="Ut")
        nc.sync.dma_start(U_t[:], U_d[:])
        R_t = const.tile([128, 896], f16, tag="R", name="Rt")
        nc.sync.dma_start(R_t[:], R_d[:])

        # ---- persistent activations ----
        QT = [[actp.tile([128, 512], f16, tag=f"qt{ct}_{sc}", name=f"qt{ct}_{sc}") for sc in range(4)]
              for ct in range(4)]
        KT = [[actp.tile([128, 512], f16, tag=f"kt{ct}_{sc}", name=f"kt{ct}_{sc}") for sc in range(4)]
              for ct in range(4)]
        V = [actp.tile([128, 8 * 65], f16, tag=f"v{st}", name=f"v{st}") for st in range(16)]
        OTn = [[actp.tile([128, 512], f16, tag=f"otn{hp}_{qc}", name=f"otn{hp}_{qc}") for qc in range(4)]
               for hp in range(4)]

        # ---- phase 1: QKV projections (k-ascending for attention overlap) ----
        for sc in range(4):
            ss = slice(sc * 512, (sc + 1) * 512)
            for ct in range(4):
                cs = slice(ct * 128, (ct + 1) * 128)
                p = ps_p.tile([128, 512], f32, tag="p512", name="p512")
                for d in range(8):
                    nc.tensor.matmul(p[:], wk[d][:, cs], xt[d][:, ss],
                                     start=(d == 0), stop=(d == 7))
                nc.vector.tensor_scalar_add(KT[ct][sc][:], p[:], bk[ct][:])
            for stl in range(4):
                st = 4 * sc + stl
                ts = slice(st * 128, (st + 1) * 128)
                p = ps_p.tile([128, 512], f32, tag="p512", name="p512")
                for d in range(8):
                    nc.tensor.matmul(p[:], xt[d][:, ts], wv[d][:, :],
                                     start=(d == 0), stop=(d == 7))
                v3 = V[st][:].rearrange("p (h e) -> p h e", e=65)
                nc.vector.scalar_tensor_tensor(
                    v3[:, :, 0:64],
                    p[:].rearrange("p (h e) -> p h e", e=64),
                    1.0,
                    bv_t[:].rearrange("p (h e) -> p h e", e=64),
                    mult, add,
                )
                nc.vector.memset(v3[:, :, 64:65], 1.0)
            for ct in range(4):
                cs = slice(ct * 128, (ct + 1) * 128)
                p = ps_p.tile([128, 512], f32, tag="p512", name="p512")
                for d in range(8):
                    nc.tensor.matmul(p[:], wq[d][:, cs], xt[d][:, ss],
                                     start=(d == 0), stop=(d == 7))
                nc.vector.tensor_scalar_add(QT[ct][sc][:], p[:], bq[ct][:])

        # ---- phase 2: attention + phase 3 out-projection, chunk by chunk ----
        for qc in range(4):
            for h in range(HPC):
                hp = h // 2
                off = (h % 2) * 64
                prow = slice(off, off + 64)
                nkt = 4 * (qc + 1)
                av = ps_p.tile([65, 512], f32, tag="p512", name="av")
                for g in range(nkt // 2):
                    sp = ps_s.tile([128, 1024], f32, tag="s2", name="s2")
                    for j in range(2):
                        kt = 2 * g + j
                        js = slice(j * 512, (j + 1) * 512)
                        diag = kt >= 4 * qc
                        nc.tensor.matmul(
                            sp[:, js],
                            KT[hp][kt // 4][prow, (kt % 4) * 128 : (kt % 4) * 128 + 128],
                            QT[hp][qc][prow, :],
                            start=True, stop=not diag, skip_group_check=True,
                        )
                        if diag:
                            delta = 128 * kt - 512 * qc
                            nc.tensor.matmul(
                                sp[:, js], U_t[:],
                                R_t[:, 384 - delta : 384 - delta + 512],
                                start=False, stop=True, skip_group_check=True,
                            )
                    pt = work.tile([128, 1024], f16, tag="pt", name="pt")
                    nc.scalar.activation(pt[:], sp[:], Exp, scale=SCALE)
                    for j in range(2):
                        kt = 2 * g + j
                        nc.tensor.matmul(
                            av[:], V[kt][:, h * 65 : h * 65 + 65],
                            pt[:, j * 512 : (j + 1) * 512],
                            start=(g == 0 and j == 0),
                            stop=(g == nkt // 2 - 1 and j == 1),
                            skip_group_check=True,
                        )
                # softmax normalization: rows 0-63 = O^T, row 64 = denom
                r = work.tile([1, 512], f32, tag="r", name="r")
                nc.vector.reciprocal(r[:], av[64:65, :])
                rb = work.tile([64, 512], f32, tag="rb", name="rb")
                nc.gpsimd.partition_broadcast(rb[:], r[:], channels=64)
                if off == 0:
                    nc.vector.tensor_mul(OTn[hp][qc][0:64, :], av[0:64, :], rb[:])
                else:
                    tmp = work.tile([64, 512], f16, tag="otmp", name="otmp")
                    nc.vector.tensor_mul(tmp[:], av[0:64, :], rb[:])
                    nc.sync.dma_start(OTn[hp][qc][64:128, :], tmp[:])
            # out-projection for the 4 seq-tiles of this chunk
            for stl in range(4):
                st = 4 * qc + stl
                sl = slice(stl * 128, (stl + 1) * 128)
                for oc in range(2):
                    ocs = slice(oc * 512, (oc + 1) * 512)
                    yp = ps_p.tile([128, 512], f32, tag="p512", name="p512")
                    for hp in range(4):
                        nc.tensor.matmul(yp[:], OTn[hp][qc][:, sl], wo[hp][:, ocs],
                                         start=(hp == 0), stop=(hp == 3),
                                         skip_group_check=True)
                    ysb = work.tile([128, 512], f32, tag="ysb", name="ysb")
                    nc.vector.scalar_tensor_tensor(ysb[:], yp[:], 1.0,
                                                   bo_t[:, ocs], mult, add)
                    nc.sync.dma_start(y_d[st * 128 : (st + 1) * 128, ocs], ysb[:])

    nc.compile()
    return nc


def _host_inputs(x, w_qkv, b_qkv, w_o, b_o):
    """Per-core input dicts implementing the sharding + layout prep."""
    U = np.zeros((128, 128), np.float16)
    for c in range(128):
        U[c, c:] = 1.0
    R = np.zeros((128, 896), np.float16)
    for c in range(128):
        R[c, : c + 384] = -BIG

    in_maps = []
    for c in range(N_CORES):
        b = c // 2
        hs = (c % 2) * HPC
        cols = slice(hs * DH, (hs + HPC) * DH)
        in_maps.append({
            "xT": np.ascontiguousarray(x[b].T).astype(np.float16),
            "wq": w_qkv[:, cols].astype(np.float16),
            "wk": w_qkv[:, D:][:, cols].astype(np.float16),
            "wv": w_qkv[:, 2 * D:][:, cols].astype(np.float16),
            "wo": w_o[hs * DH : (hs + HPC) * DH, :].astype(np.float16),
            "bq": b_qkv[cols].reshape(CH, 1).astype(np.float32),
            "bk": b_qkv[D:][cols].reshape(CH, 1).astype(np.float32),
            "bvb": np.tile(b_qkv[2 * D:][cols].astype(np.float32), (128, 1)),
            "bob": np.tile(b_o.astype(np.float32), (128, 1)),
            "U": U,
            "R": R,
        })
    return in_maps


def kernel(x, w_qkv, b_qkv, w_o, b_o):
    global _cached
    from concourse.bass_utils import run_bass_kernel_spmd

    x = np.asarray(x)
    w_qkv = np.asarray(w_qkv)
    b_qkv = np.asarray(b_qkv)
    w_o = np.asarray(w_o)
    b_o = np.asarray(b_o)

    if _cached is None:
        _cached = _build_program()
    nc = _cached

    in_maps = _host_inputs(x, w_qkv, b_qkv, w_o, b_o)
    res = run_bass_kernel_spmd(nc, in_maps, list(range(N_CORES)))

    out = np.empty((B, N, D), np.float32)
    for b in range(B):
        out[b] = res.results[2 * b]["y"] + res.results[2 * b + 1]["y"]
    return out
